# revision 1
# baseline (speedup 1.0000x reference)
import os
import sys

sys.path.insert(0, "/opt/trn_rl_repo")

import numpy as np

import concourse.bacc as bacc
import concourse.bass as bass
import concourse.mybir as mybir
from concourse.tile import TileContext
from concourse.bass_utils import run_bass_kernel_spmd

# Problem constants (hardcoded from spec)
E, G, TOPK = 32, 16, 2
HID, INTER, A_INTER = 1024, 2048, 128
CAP_FACTOR = 1.25
SCALE = 0.05
B, N = 4, 1024
T = B * N                      # 4096 tokens
CAP = int(CAP_FACTOR * T / E)  # 160
NCORES = 8
E_LOC = E // NCORES            # 4 experts per core
G_LOC = G // NCORES            # 2 adjugate groups per core

F32 = mybir.dt.float32
DT = mybir.dt.bfloat16         # matmul dtype (float32 or bfloat16)

LAST_EXEC_NS = None

_cache = {}


def _gelu(x):
    from scipy.special import erf
    return (0.5 * x * (1.0 + erf(x / np.float32(np.sqrt(2.0))))).astype(np.float32)


def _route(x, r1_w, r1_b, r2_w):
    """Numpy float32 routing that mirrors reference.py exactly."""
    xf = x.reshape(-1, HID).astype(np.float32)
    mean = xf.mean(-1, keepdims=True, dtype=np.float32)
    std = xf.std(-1, ddof=1, keepdims=True).astype(np.float32)
    mn = xf.min(-1, keepdims=True)
    mx = xf.max(-1, keepdims=True)
    l2 = np.sqrt((xf * xf).sum(-1, keepdims=True, dtype=np.float32))
    sp = (np.abs(xf) < 1e-6).astype(np.float32).mean(-1, keepdims=True, dtype=np.float32)
    ri = np.concatenate([xf, mean, std, mn, mx, l2, sp], -1)

    h = _gelu(ri @ r1_w.T + r1_b)
    logits = h @ r2_w.T
    logits = logits - logits.max(-1, keepdims=True)
    p = np.exp(logits)
    probs = p / p.sum(-1, keepdims=True)                      # [T, E]

    order = np.argsort(-probs, axis=-1, kind="stable")
    topi = order[:, :TOPK]                                    # [T, K]
    topp = np.take_along_axis(probs, topi, axis=-1)
    wnorm = topp / topp.sum(-1, keepdims=True)

    eids = np.arange(E)
    hit = topi[..., None] == eids                             # [T, K, E]
    routed = hit.any(1)                                       # [T, E]
    Wc = np.where(hit, wnorm[..., None], 0.0).sum(1).astype(np.float32)  # [T, E]

    score = np.where(routed, probs, -np.inf)
    idx = np.argsort(-score, axis=0, kind="stable")[:CAP].T   # [E, cap]
    valid = np.take_along_axis(routed.T, idx, 1)              # [E, cap]
    w = (np.take_along_axis(Wc.T, idx, 1) * valid).astype(np.float32)  # [E, cap]

    Wmask = np.zeros((T, E), np.float32)
    for e in range(E):
        Wmask[idx[e], e] += w[e]
    gw = (SCALE * Wmask.reshape(T, G, E // G).sum(-1)).astype(np.float32)  # [T, G]
    return xf, idx.astype(np.int64), w, gw


def _build_device_program():
    nc = bacc.Bacc(None, target_bir_lowering=False, debug=True, detect_race_conditions=True)

    xe_d = nc.dram_tensor("xe", [E_LOC, 128, 8 * CAP], DT, kind="ExternalInput")
    wu_d = nc.dram_tensor("wu", [E_LOC, 2 * INTER // 128, 128, 8 * 128], DT, kind="ExternalInput")
    wd_d = nc.dram_tensor("wd", [E_LOC, HID // 128, 128, INTER], DT, kind="ExternalInput")
    wb_d = nc.dram_tensor("wb", [E_LOC, 128, CAP], F32, kind="ExternalInput")
    xt_d = nc.dram_tensor("xt", [8, 128, T], DT, kind="ExternalInput")
    au_d = nc.dram_tensor("au", [G_LOC, 128, 8 * 2 * A_INTER], DT, kind="ExternalInput")
    ad_d = nc.dram_tensor("ad", [G_LOC, A_INTER, HID], DT, kind="ExternalInput")
    gwb_d = nc.dram_tensor("gwb", [G_LOC, 128, T], F32, kind="ExternalInput")

    ye_d = nc.dram_tensor("ye", [E_LOC, 8, 128, CAP], F32, kind="ExternalOutput")
    adj_d = nc.dram_tensor("adj", [8, 128, T], F32, kind="ExternalOutput")

    NJC = 2 * INTER // 128    # 32 up column-chunks (16 gate + 16 upv)
    NJH = NJC // 2            # 16
    TC = 512                  # adjugate token chunk
    NTC = T // TC             # 8

    with TileContext(nc) as tc:
        with (
            tc.tile_pool(name="xe_p", bufs=2) as xe_p,
            tc.tile_pool(name="wb_p", bufs=2) as wb_p,
            tc.tile_pool(name="wu_p", bufs=12) as wu_p,
            tc.tile_pool(name="wd_p", bufs=4) as wd_p,
            tc.tile_pool(name="act_p", bufs=2) as act_p,
            tc.tile_pool(name="tmp_p", bufs=4) as tmp_p,
            tc.tile_pool(name="out_p", bufs=6) as out_p,
            tc.tile_pool(name="au_p", bufs=1) as au_p,
            tc.tile_pool(name="ad_p", bufs=1) as ad_p,
            tc.tile_pool(name="xt_p", bufs=18) as xt_p,
            tc.tile_pool(name="gw_p", bufs=6) as gw_p,
            tc.tile_pool(name="aact_p", bufs=3) as aact_p,
            tc.tile_pool(name="ps_up", bufs=3, space="PSUM") as ps_up,
            tc.tile_pool(name="ps_dn", bufs=2, space="PSUM") as ps_dn,
        ):
            au_t = []
            ad_t = []
            for g in range(G_LOC):
                t1 = au_p.tile([128, 8 * 2 * A_INTER], DT, tag=f"au{g}")
                nc.gpsimd.dma_start(out=t1[:], in_=au_d[g])
                au_t.append(t1)
                t2 = ad_p.tile([128, HID], DT, tag=f"ad{g}")
                nc.gpsimd.dma_start(out=t2[:], in_=ad_d[g])
                ad_t.append(t2)

            acts = {}

            def emit_up(e):
                xe_t = xe_p.tile([128, 8 * CAP], DT, tag="xe")
                nc.gpsimd.dma_start(out=xe_t[:], in_=xe_d[e])
                wb_t = wb_p.tile([128, CAP], F32, tag="wb")
                nc.gpsimd.dma_start(out=wb_t[:], in_=wb_d[e])
                act_t = act_p.tile([128, NJH * CAP], DT, tag="act")
                acts[e] = act_t
                for jc in range(NJH):
                    wug = wu_p.tile([128, 8 * 128], DT, tag="wu")
                    nc.sync.dma_start(out=wug[:], in_=wu_d[e, jc])
                    wuu = wu_p.tile([128, 8 * 128], DT, tag="wu")
                    nc.scalar.dma_start(out=wuu[:], in_=wu_d[e, jc + NJH])
                    ps_g = ps_up.tile([128, CAP], F32, tag="psg")
                    ps_u = ps_up.tile([128, CAP], F32, tag="psu")
                    for kc in range(8):
                        nc.tensor.matmul(
                            ps_g[:], lhsT=wug[:, kc * 128:(kc + 1) * 128],
                            rhs=xe_t[:, kc * CAP:(kc + 1) * CAP],
                            start=(kc == 0), stop=(kc == 7))
                    for kc in range(8):
                        nc.tensor.matmul(
                            ps_u[:], lhsT=wuu[:, kc * 128:(kc + 1) * 128],
                            rhs=xe_t[:, kc * CAP:(kc + 1) * CAP],
                            start=(kc == 0), stop=(kc == 7))
                    tmp = tmp_p.tile([128, CAP], F32, tag="tmp")
                    nc.scalar.activation(tmp[:], ps_g[:], mybir.ActivationFunctionType.Sigmoid)
                    nc.vector.tensor_mul(tmp[:], tmp[:], ps_g[:])
                    nc.vector.tensor_mul(tmp[:], tmp[:], ps_u[:])
                    nc.vector.tensor_mul(act_t[:, jc * CAP:(jc + 1) * CAP], tmp[:], wb_t[:])

            def emit_down(e):
                act_t = acts.pop(e)
                for oc in range(8):
                    wdt = wd_p.tile([128, INTER], DT, tag="wd")
                    (nc.sync if oc % 2 == 0 else nc.scalar).dma_start(out=wdt[:], in_=wd_d[e, oc])
                    ps_d = ps_dn.tile([128, CAP], F32, tag="psd")
                    for jc in range(NJH):
                        nc.tensor.matmul(
                            ps_d[:], lhsT=wdt[:, jc * 128:(jc + 1) * 128],
                            rhs=act_t[:, jc * CAP:(jc + 1) * CAP],
                            start=(jc == 0), stop=(jc == NJH - 1))
                    ot = out_p.tile([128, CAP], F32, tag="oexp")
                    nc.scalar.copy(ot[:], ps_d[:])
                    nc.gpsimd.dma_start(out=ye_d[e, oc], in_=ot[:])

            def emit_adj(tci):
                xts = []
                for kc in range(8):
                    xt_t = xt_p.tile([128, TC], DT, tag="xt")
                    (nc.sync if kc % 2 == 0 else nc.scalar).dma_start(
                        out=xt_t[:], in_=xt_d[kc, :, tci * TC:(tci + 1) * TC])
                    xts.append(xt_t)
                aacts = []
                for g in range(G_LOC):
                    gw_t = gw_p.tile([128, TC], F32, tag="gw")
                    nc.gpsimd.dma_start(out=gw_t[:], in_=gwb_d[g, :, tci * TC:(tci + 1) * TC])
                    ps_ag = ps_up.tile([128, TC], F32, tag="psg")
                    ps_au = ps_up.tile([128, TC], F32, tag="psu")
                    for kc in range(8):
                        nc.tensor.matmul(
                            ps_ag[:], lhsT=au_t[g][:, kc * 256:kc * 256 + 128],
                            rhs=xts[kc][:], start=(kc == 0), stop=(kc == 7))
                    for kc in range(8):
                        nc.tensor.matmul(
                            ps_au[:], lhsT=au_t[g][:, kc * 256 + 128:kc * 256 + 256],
                            rhs=xts[kc][:], start=(kc == 0), stop=(kc == 7))
                    aact = aact_p.tile([128, TC], DT, tag="aact")
                    tmpa = aact_p.tile([128, TC], F32, tag="tmpa")
                    nc.scalar.activation(tmpa[:], ps_ag[:], mybir.ActivationFunctionType.Sigmoid)
                    nc.vector.tensor_mul(tmpa[:], tmpa[:], ps_ag[:])
                    nc.vector.tensor_mul(tmpa[:], tmpa[:], ps_au[:])
                    nc.vector.tensor_mul(aact[:], tmpa[:], gw_t[:])
                    aacts.append(aact)
                for oc in range(8):
                    ps_adj = ps_dn.tile([128, TC], F32, tag="psd")
                    for g in range(G_LOC):
                        nc.tensor.matmul(
                            ps_adj[:], lhsT=ad_t[g][:, oc * 128:(oc + 1) * 128],
                            rhs=aacts[g][:], start=(g == 0), stop=(g == G_LOC - 1))
                    oadj = out_p.tile([128, TC], F32, tag="oadj")
                    nc.scalar.copy(oadj[:], ps_adj[:])
                    nc.gpsimd.dma_start(out=adj_d[oc, :, tci * TC:(tci + 1) * TC], in_=oadj[:])

            sched = [("u", 0), ("u", 1), ("d", 0), ("a", 0), ("u", 2), ("d", 1),
                     ("a", 1), ("u", 3), ("d", 2), ("a", 2), ("d", 3), ("a", 3),
                     ("a", 4), ("a", 5), ("a", 6), ("a", 7)]
            for kind, i in sched:
                if kind == "u":
                    emit_up(i)
                elif kind == "d":
                    emit_down(i)
                else:
                    emit_adj(i)

    nc.finalize()
    return nc


def _np_dt(a):
    if DT == mybir.dt.float32:
        return np.ascontiguousarray(a, dtype=np.float32)
    import ml_dtypes
    return np.ascontiguousarray(a.astype(ml_dtypes.bfloat16))


def kernel(x, r1_w, r1_b, r2_w, w_up, w_down, a_up, a_down):
    global LAST_EXEC_NS
    x = np.asarray(x, np.float32)
    r1_w = np.asarray(r1_w, np.float32)
    r1_b = np.asarray(r1_b, np.float32)
    r2_w = np.asarray(r2_w, np.float32)
    w_up = np.asarray(w_up, np.float32)
    w_down = np.asarray(w_down, np.float32)
    a_up = np.asarray(a_up, np.float32)
    a_down = np.asarray(a_down, np.float32)

    xf, idx, w, gw = _route(x, r1_w, r1_b, r2_w)

    # weight layouts (per-expert column slabs, contiguous for DMA)
    if "wu" not in _cache:
        w_upT = w_up.transpose(0, 2, 1)                          # [E, HID, 2I]
        _cache["wu"] = np.ascontiguousarray(
            w_upT.reshape(E, 8, 128, 32, 128).transpose(0, 3, 2, 1, 4)
            .reshape(E, 32, 128, 8 * 128))                       # [E, 32, 128, 1024]
        w_downT = w_down.transpose(0, 2, 1)                      # [E, I, HID]
        _cache["wd"] = np.ascontiguousarray(
            w_downT.reshape(E, 16, 128, 8, 128).transpose(0, 3, 2, 1, 4)
            .reshape(E, 8, 128, INTER))                          # [E, 8, 128, 2048]
        _cache["au"] = np.ascontiguousarray(
            a_up.transpose(0, 2, 1).reshape(G, 8, 128, 2 * A_INTER)
            .transpose(0, 2, 1, 3).reshape(G, 128, 8 * 2 * A_INTER))
        _cache["ad"] = np.ascontiguousarray(a_down.transpose(0, 2, 1))  # [G, A_I, HID]
        _cache["wu"] = _np_dt(_cache["wu"])
        _cache["wd"] = _np_dt(_cache["wd"])
        _cache["au"] = _np_dt(_cache["au"])
        _cache["ad"] = _np_dt(_cache["ad"])
    wu, wd, au, ad = _cache["wu"], _cache["wd"], _cache["au"], _cache["ad"]

    xT = _np_dt(xf.T.reshape(8, 128, T))

    in_maps = []
    for c in range(NCORES):
        es = slice(c * E_LOC, (c + 1) * E_LOC)
        gs = slice(c * G_LOC, (c + 1) * G_LOC)
        xe = xf[idx[es]]                                          # [4, cap, HID]
        xe = _np_dt(xe.transpose(0, 2, 1).reshape(E_LOC, 8, 128, CAP)
                    .transpose(0, 2, 1, 3).reshape(E_LOC, 128, 8 * CAP))
        wb = np.ascontiguousarray(
            np.broadcast_to(w[es][:, None, :], (E_LOC, 128, CAP)), np.float32)
        gwb = np.ascontiguousarray(
            np.broadcast_to(gw.T[gs][:, None, :], (G_LOC, 128, T)), np.float32)
        in_maps.append({
            "xe": xe, "wu": wu[es], "wd": wd[es], "wb": wb,
            "xt": xT, "au": au[gs], "ad": ad[gs], "gwb": gwb,
        })

    if "nc" not in _cache:
        _cache["nc"] = _build_device_program()
    nc = _cache["nc"]

    res = run_bass_kernel_spmd(nc, in_maps, list(range(NCORES)))
    LAST_EXEC_NS = res.exec_time_ns

    out = np.zeros((T, HID), np.float32)
    for c in range(NCORES):
        out += res.results[c]["adj"].reshape(HID, T).T
    for e in range(E):
        c = e // E_LOC
        ye = res.results[c]["ye"][e % E_LOC].reshape(HID, CAP)    # [HID, cap]
        out[idx[e]] += ye.T
    return out.reshape(B, N, HID)

